# revision 19
# baseline (speedup 1.0000x reference)
"""Trainium2 Bass kernel for nn_EnsembleEnvProb (ensemble probabilistic MLP).

Math (per ensemble member e, batch row n):
  x = concat(s, a)                                   [38]
  h = swish(x @ W1[e] + b1[e]); 3x swish(h @ Wh[l,e] + bh[l,e])
  mu    = h @ Wmu[e] + bmu[e]                        [31]
  z     = h @ Wsig[e] + bsig[e]                      [31]  (raw logvar/2)
  soft-clamp z into [min_lv, max_lv] via softplus, sigma = exp(z')

Identity used on-chip (no softplus table exists on TRN2):
  g = sigmoid(2z - max) = 0.5 + 0.5*tanh(z - max/2)
  sigma^2 = exp(min) + 0.5*exp(max)*(1 + tanh(z - max/2))
  sigma = Sqrt(k1*t + k2), k1 = 0.5*exp(max), k2 = exp(min) + k1

Sharding: batch N split across 8 NeuronCores (data parallel, weights
replicated). Host pre-transposes activations to [E, feat, N] so all
on-chip tensors are feature-major (features on SBUF partitions, samples
on the free dim) and every DMA is wide and contiguous.

Per core (N_loc = N/8), phase 1 (silu ACT table set) iterates over
(e, pair-of-512-sample tiles): float32r matmuls with K split 128+72
accumulating in PSUM, hidden M split 128+72; the two tiles of a pair
share PSUM tiles side by side so every ScalarE op runs at FD=1024 with
a correct per-partition bias. mu leaves directly; tanh(z - max/2) goes
to a DRAM scratch. Phase 2 (exp/ln set) finishes sigma.
"""

import os
from contextlib import ExitStack

import numpy as np

import concourse.bass as bass
import concourse.mybir as mybir
import concourse.tile as tile
from concourse import bacc
from concourse.bass_utils import run_bass_kernel_spmd

F32 = mybir.dt.float32
F32R = mybir.dt.float32r
AF = mybir.ActivationFunctionType
ALU = mybir.AluOpType

ENS = 7
DIN = 38
HID = 200
NSIG = 31          # outputs per head (30 ds + 1 r)
NOUT = 64          # sigma head [0:31], pad, mu head [32:63], pad
NCORES = 8
T = 512            # samples per matmul pass (fp32 moving-operand max)
GRP = 1024         # samples per chain step (PSUM tile free dim)
IL = 2             # chains issued in lockstep
P2C = 2048         # phase-2 free-dim chunk

_cache = {}


def _build(npc, grp=GRP, il=IL):
    """Build + compile the per-core Bass program for npc samples/core."""
    nstep = npc // (grp * il)
    ppbufs = max(2, 2 * (8 // (2 * (grp // T))) // 1)
    ppbufs = 8 // (grp // T)
    nc = bacc.Bacc("TRN2", target_bir_lowering=False, debug=False,
                   num_devices=NCORES)

    xT = nc.dram_tensor("xT", [ENS, DIN, npc], F32R, kind="ExternalInput").ap()
    w1 = nc.dram_tensor("w1", [ENS, DIN, HID], F32R, kind="ExternalInput").ap()
    wh = nc.dram_tensor("wh", [3, ENS, HID, HID], F32R,
                        kind="ExternalInput").ap()
    wo = nc.dram_tensor("wo", [ENS, HID, NOUT], F32R,
                        kind="ExternalInput").ap()
    b1T = nc.dram_tensor("b1T", [HID, ENS], F32, kind="ExternalInput").ap()
    bhT = nc.dram_tensor("bhT", [3, HID, ENS], F32, kind="ExternalInput").ap()
    bmuT = nc.dram_tensor("bmuT", [128, ENS], F32, kind="ExternalInput").ap()
    tbT = nc.dram_tensor("tbT", [128, ENS], F32, kind="ExternalInput").ap()
    k1v = nc.dram_tensor("k1v", [4 * NSIG, 1], F32,
                         kind="ExternalInput").ap()
    k2v = nc.dram_tensor("k2v", [4 * NSIG, 1], F32,
                         kind="ExternalInput").ap()

    mu_out = nc.dram_tensor("mu_out", [ENS, NSIG, npc], F32,
                            kind="ExternalOutput").ap()
    sg_out = nc.dram_tensor("sg_out", [ENS, NSIG, npc], F32,
                            kind="ExternalOutput").ap()
    gdram = nc.dram_tensor("gscratch", [ENS, NSIG, npc], F32).ap()

    with tile.TileContext(nc) as tc, ExitStack() as ctx:
        wp = ctx.enter_context(tc.tile_pool(name="wp", bufs=1))
        iop = ctx.enter_context(tc.tile_pool(name="iop", bufs=2 * il + 1))
        p2p = ctx.enter_context(tc.tile_pool(name="p2p", bufs=3))
        p2o = ctx.enter_context(tc.tile_pool(name="p2o", bufs=2))
        pp = ctx.enter_context(tc.tile_pool(name="pp", bufs=ppbufs,
                                            space="PSUM"))

        # ---- resident weights + biases ----
        # Weight DMAs are issued inside the phase-1 e loop (just before
        # first use) so the first chains are not stuck behind the whole
        # 3.9 MB preload.
        w1_sb, whA_sb, whB_sb, wo_sb = {}, {}, {}, {}

        def load_weights(e):
            w1_sb[e] = wp.tile([DIN, HID], F32R, tag=f"w1_{e}",
                               name=f"w1_{e}")
            nc.sync.dma_start(w1_sb[e], w1[e])
            for l in range(3):
                whA_sb[l, e] = wp.tile([128, HID], F32R, tag=f"whA_{l}_{e}",
                                       name=f"whA_{l}_{e}")
                whB_sb[l, e] = wp.tile([72, HID], F32R, tag=f"whB_{l}_{e}",
                                       name=f"whB_{l}_{e}")
                nc.sync.dma_start(whA_sb[l, e], wh[l, e, 0:128, :])
                nc.sync.dma_start(whB_sb[l, e], wh[l, e, 128:HID, :])
            woA = wp.tile([128, NOUT], F32R, tag=f"woA_{e}", name=f"woA_{e}")
            woB = wp.tile([72, NOUT], F32R, tag=f"woB_{e}", name=f"woB_{e}")
            nc.sync.dma_start(woA, wo[e, 0:128, :])
            nc.sync.dma_start(woB, wo[e, 128:HID, :])
            wo_sb[e] = (woA, woB)

        b1A = wp.tile([128, ENS], F32, tag="b1A", name="b1A")
        b1B = wp.tile([72, ENS], F32, tag="b1B", name="b1B")
        nc.sync.dma_start(b1A, b1T[0:128, :])
        nc.sync.dma_start(b1B, b1T[128:HID, :])
        bhA, bhB = {}, {}
        for l in range(3):
            bhA[l] = wp.tile([128, ENS], F32, tag=f"bhA{l}", name=f"bhA{l}")
            bhB[l] = wp.tile([72, ENS], F32, tag=f"bhB{l}", name=f"bhB{l}")
            nc.sync.dma_start(bhA[l], bhT[l, 0:128, :])
            nc.sync.dma_start(bhB[l], bhT[l, 128:HID, :])
        bmu2_sb = wp.tile([128, ENS], F32, tag="bmu", name="bmu")
        tb2_sb = wp.tile([128, ENS], F32, tag="tb", name="tb")
        nc.sync.dma_start(bmu2_sb, bmuT)
        nc.sync.dma_start(tb2_sb, tbT)
        k1_sb = wp.tile([4 * NSIG, 1], F32, tag="k1", name="k1")
        k2_sb = wp.tile([4 * NSIG, 1], F32, tag="k2", name="k2")
        nc.sync.dma_start(k1_sb, k1v)
        nc.sync.dma_start(k2_sb, k2v)

        # ---- phase 1: MLP chain, silu table set ----
        # Each PSUM tile is [128, grp]: the grp//T matmul passes of a
        # chain step sit side by side, so ScalarE ops span FD=grp with a
        # per-partition bias that is still exact. `il` chains are issued
        # in lockstep so another chain's matmuls cover each silu latency.
        halves = tuple(slice(i * T, (i + 1) * T) for i in range(grp // T))

        def mlp_layer(lhsA, lhsB, biasA, biasB, rhsA, rhsB, e):
            """One layer for one chain step. lhsA/lhsB: [128,200]/[72,200]
            weight K-halves (lhsB unused for the input layer); rhsA/rhsB:
            previous h as ([K1, grp], [K2, grp]) (rhsB None on input)."""
            psM1 = pp.tile([128, grp], F32, tag="ps", name="psM1")
            psM2 = pp.tile([128, grp], F32, tag="ps", name="psM2")
            for h in halves:
                nc.tensor.matmul(psM1[:, h], lhsT=lhsA[:, 0:128],
                                 rhs=rhsA[:, h], start=True, stop=rhsB is None)
                if rhsB is not None:
                    nc.tensor.matmul(psM1[:, h], lhsT=lhsB[:, 0:128],
                                     rhs=rhsB[:, h], start=False, stop=True)
                nc.tensor.matmul(psM2[0:72, h], lhsT=lhsA[:, 128:HID],
                                 rhs=rhsA[:, h], start=True, stop=rhsB is None)
                if rhsB is not None:
                    nc.tensor.matmul(psM2[0:72, h], lhsT=lhsB[:, 128:HID],
                                     rhs=rhsB[:, h], start=False, stop=True)
            hM1 = iop.tile([128, grp], F32R, tag="hM1", name="hM1")
            hM2 = iop.tile([72, grp], F32R, tag="hM2", name="hM2")
            nc.scalar.activation(hM1, psM1, AF.Silu, bias=biasA[:, e:e + 1])
            nc.scalar.activation(hM2, psM2[0:72, :], AF.Silu,
                                 bias=biasB[:, e:e + 1])
            return hM1, hM2

        warm = pp.tile([128, grp], F32, tag="ps", name="warm")
        load_weights(0)
        for i in range(8):
            nc.tensor.matmul(warm[:, 0:HID], lhsT=w1_sb[0][:, 0:128],
                             rhs=w1_sb[0], start=True, stop=True,
                             skip_group_check=True)

        for e in range(ENS):
            if e > 0:
                load_weights(e)
            for p in range(nstep):
                slots = [bass.ts(p * il + i, grp) for i in range(il)]
                xs = []
                for cs in slots:
                    x_sb = iop.tile([DIN, grp], F32R, tag="x", name="x")
                    nc.sync.dma_start(x_sb, xT[e, :, cs])
                    xs.append(x_sb)

                hs = [mlp_layer(w1_sb[e], None, b1A, b1B, x_sb, None, e)
                      for x_sb in xs]
                for l in range(3):
                    hs = [mlp_layer(whA_sb[l, e], whB_sb[l, e],
                                    bhA[l], bhB[l], hM1, hM2, e)
                          for hM1, hM2 in hs]

                # Per chain: psO rows 0:31 sig, 32:63 mu. (f32r matmuls
                # may only write PSUM partition base 0; PSUM reads with
                # base>0 may span at most 32 partitions.)
                woA, woB = wo_sb[e]
                for cs, (hM1, hM2) in zip(slots, hs):
                    psO = pp.tile([128, grp], F32, tag="ps", name="psO")
                    for h in halves:
                        nc.tensor.matmul(psO[0:NOUT, h], lhsT=woA,
                                         rhs=hM1[:, h], start=True, stop=False)
                        nc.tensor.matmul(psO[0:NOUT, h], lhsT=woB,
                                         rhs=hM2[:, h], start=False, stop=True)

                    gt = iop.tile([NSIG, grp], F32, tag="gt", name="gt")
                    nc.scalar.activation(gt, psO[0:NSIG, :], AF.Tanh,
                                         bias=tb2_sb[0:NSIG, e:e + 1])
                    wv = iop.tile([NSIG, grp], F32, tag="wv1", name="wv1")
                    nc.vector.tensor_scalar(wv, gt, k1_sb[0:NSIG, 0:1],
                                            k2_sb[0:NSIG, 0:1],
                                            ALU.mult, ALU.add)
                    nc.sync.dma_start(gdram[e, :, cs], wv)

                    mu_sb = iop.tile([NSIG, grp], F32, tag="mu", name="mu")
                    nc.vector.tensor_scalar_add(mu_sb, psO[32:32 + NSIG, :],
                                                bmu2_sb[32:32 + NSIG,
                                                        e:e + 1])
                    nc.sync.dma_start(mu_out[e, :, cs], mu_sb)

        # ---- phase 2: sigma = sqrt(w) from the w scratch, sqrt set ----
        # Hard scheduling barrier: phase-2 Sqrt must not interleave with
        # phase-1 Silu/Tanh, or the ACT table set thrashes (~1.3us/reload).
        # Ensembles are packed 4-at-a-time onto partitions (124 lanes).
        tc.strict_bb_all_engine_barrier()
        for e0, ne in ((0, 4), (4, 3)):
            P = ne * NSIG
            for c in range(npc // P2C):
                ps = bass.ts(c, P2C)
                gin = p2p.tile([4 * NSIG, P2C], F32, tag="gin", name="gin")
                for j in range(ne):
                    nc.sync.dma_start(gin[j * NSIG:(j + 1) * NSIG, :],
                                      gdram[e0 + j, :, ps])
                sg = p2o.tile([4 * NSIG, P2C], F32, tag="sg", name="sg")
                nc.scalar.activation(sg[0:P], gin[0:P], AF.Sqrt)
                for j in range(ne):
                    nc.sync.dma_start(sg_out[e0 + j, :, ps],
                                      sg[j * NSIG:(j + 1) * NSIG, :])

    nc.compile()
    return nc


def kernel(s, a, W1, b1, Wh, bh, Wmu, bmu, Wsig, bsig,
           max_lv_s, min_lv_s, max_lv_r, min_lv_r):
    N = s.shape[0]
    npc = N // NCORES

    f = np.float32
    x = np.concatenate([np.asarray(s, f), np.asarray(a, f)], axis=-1)
    xT = np.ascontiguousarray(np.transpose(x, (1, 2, 0)))      # [E, 38, N]

    maxv = np.concatenate([np.asarray(max_lv_s, f)[0],
                           np.asarray(max_lv_r, f)[0]])        # [31]
    minv = np.concatenate([np.asarray(min_lv_s, f)[0],
                           np.asarray(min_lv_r, f)[0]])        # [31]

    z1 = np.zeros((ENS, HID, 1), f)
    wo = np.ascontiguousarray(
        np.concatenate([np.asarray(Wsig, f), z1, np.asarray(Wmu, f), z1],
                       axis=2))
    b1T = np.ascontiguousarray(np.asarray(b1, f).T)            # [200, 7]
    bhT = np.ascontiguousarray(np.transpose(np.asarray(bh, f), (0, 2, 1)))
    tbT = np.zeros((128, ENS), f)           # tanh bias at rows 0:31, 64:95
    tbT[0:31] = np.asarray(bsig, f).T - maxv[:, None] / 2.0
    tbT[64:95] = tbT[0:31]
    bmuT = np.zeros((128, ENS), f)          # mu bias at rows 32:63, 96:127
    bmuT[32:63] = np.asarray(bmu, f).T
    bmuT[96:127] = bmuT[32:63]
    k1 = (0.5 * np.exp(maxv.astype(np.float64)))[:, None]
    k2 = np.tile((np.exp(minv.astype(np.float64))[:, None] + k1), (4, 1))
    k2 = k2.astype(f)
    k1 = np.tile(k1, (4, 1)).astype(f)

    key = npc
    if key not in _cache:
        _cache[key] = _build(npc)
    nc = _cache[key]

    common = dict(w1=np.ascontiguousarray(np.asarray(W1, f)),
                  wh=np.ascontiguousarray(np.asarray(Wh, f)),
                  wo=wo, b1T=b1T, bhT=bhT, bmuT=bmuT, tbT=tbT,
                  k1v=np.ascontiguousarray(k1), k2v=np.ascontiguousarray(k2))
    in_maps = []
    for c in range(NCORES):
        m = dict(common)
        m["xT"] = np.ascontiguousarray(xT[:, :, c * npc:(c + 1) * npc])
        in_maps.append(m)

    trace = os.environ.get("BASS_KERNEL_TRACE", "0") == "1"
    res = run_bass_kernel_spmd(nc, in_maps, list(range(NCORES)), trace=trace)
    kernel.last_results = res

    mu = np.concatenate([r["mu_out"] for r in res.results], axis=2)
    sg = np.concatenate([r["sg_out"] for r in res.results], axis=2)
    mu = np.transpose(mu, (2, 0, 1))                           # [N, E, 31]
    sg = np.transpose(sg, (2, 0, 1))

    ds_mu = np.ascontiguousarray(mu[:, :, :30])
    r_mu = np.ascontiguousarray(mu[:, :, 30:31])
    ds_sg = np.ascontiguousarray(sg[:, :, :30])
    r_sg = np.ascontiguousarray(sg[:, :, 30:31])
    return ((ds_mu, ds_sg), (r_mu, r_sg))


# revision 20
# speedup vs baseline: 1.0004x; 1.0004x over previous
"""Trainium2 Bass kernel for nn_EnsembleEnvProb (ensemble probabilistic MLP).

Math (per ensemble member e, batch row n):
  x = concat(s, a)                                   [38]
  h = swish(x @ W1[e] + b1[e]); 3x swish(h @ Wh[l,e] + bh[l,e])
  mu    = h @ Wmu[e] + bmu[e]                        [31]
  z     = h @ Wsig[e] + bsig[e]                      [31]  (raw logvar/2)
  soft-clamp z into [min_lv, max_lv] via softplus, sigma = exp(z')

Identity used on-chip (no softplus table exists on TRN2):
  g = sigmoid(2z - max) = 0.5 + 0.5*tanh(z - max/2)
  sigma^2 = exp(min) + 0.5*exp(max)*(1 + tanh(z - max/2))
  sigma = Sqrt(k1*t + k2), k1 = 0.5*exp(max), k2 = exp(min) + k1

Sharding: batch N split across 8 NeuronCores (data parallel, weights
replicated). Host pre-transposes activations to [E, feat, N] so all
on-chip tensors are feature-major (features on SBUF partitions, samples
on the free dim) and every DMA is wide and contiguous.

Per core (N_loc = N/8), phase 1 (silu ACT table set) iterates over
(e, pair-of-512-sample tiles): float32r matmuls with K split 128+72
accumulating in PSUM, hidden M split 128+72; the two tiles of a pair
share PSUM tiles side by side so every ScalarE op runs at FD=1024 with
a correct per-partition bias. mu leaves directly; tanh(z - max/2) goes
to a DRAM scratch. Phase 2 (exp/ln set) finishes sigma.
"""

import os
from contextlib import ExitStack

import numpy as np

import concourse.bass as bass
import concourse.mybir as mybir
import concourse.tile as tile
from concourse import bacc
from concourse.bass_utils import run_bass_kernel_spmd

F32 = mybir.dt.float32
F32R = mybir.dt.float32r
AF = mybir.ActivationFunctionType
ALU = mybir.AluOpType

ENS = 7
DIN = 38
HID = 200
NSIG = 31          # outputs per head (30 ds + 1 r)
NOUT = 64          # sigma head [0:31], pad, mu head [32:63], pad
NCORES = 8
T = 512            # samples per matmul pass (fp32 moving-operand max)
GRP = 1024         # samples per chain step (PSUM tile free dim)
IL = 2             # chains issued in lockstep
P2C = 2048         # phase-2 free-dim chunk

_cache = {}


def _build(npc, grp=GRP, il=IL):
    """Build + compile the per-core Bass program for npc samples/core."""
    nstep = npc // (grp * il)
    ppbufs = max(2, 2 * (8 // (2 * (grp // T))) // 1)
    ppbufs = 8 // (grp // T)
    nc = bacc.Bacc("TRN2", target_bir_lowering=False, debug=False,
                   num_devices=NCORES)

    xT = nc.dram_tensor("xT", [ENS, DIN, npc], F32R, kind="ExternalInput").ap()
    w1 = nc.dram_tensor("w1", [ENS, DIN, HID], F32R, kind="ExternalInput").ap()
    wh = nc.dram_tensor("wh", [3, ENS, HID, HID], F32R,
                        kind="ExternalInput").ap()
    wo = nc.dram_tensor("wo", [ENS, HID, NOUT], F32R,
                        kind="ExternalInput").ap()
    b1T = nc.dram_tensor("b1T", [HID, ENS], F32, kind="ExternalInput").ap()
    bhT = nc.dram_tensor("bhT", [3, HID, ENS], F32, kind="ExternalInput").ap()
    bmuT = nc.dram_tensor("bmuT", [128, ENS], F32, kind="ExternalInput").ap()
    tbT = nc.dram_tensor("tbT", [128, ENS], F32, kind="ExternalInput").ap()
    k1v = nc.dram_tensor("k1v", [4 * NSIG, 1], F32,
                         kind="ExternalInput").ap()
    k2v = nc.dram_tensor("k2v", [4 * NSIG, 1], F32,
                         kind="ExternalInput").ap()

    mu_out = nc.dram_tensor("mu_out", [ENS, NSIG, npc], F32,
                            kind="ExternalOutput").ap()
    sg_out = nc.dram_tensor("sg_out", [ENS, NSIG, npc], F32,
                            kind="ExternalOutput").ap()
    gdram = nc.dram_tensor("gscratch", [ENS, NSIG, npc], F32).ap()

    with tile.TileContext(nc) as tc, ExitStack() as ctx:
        wp = ctx.enter_context(tc.tile_pool(name="wp", bufs=1))
        iop = ctx.enter_context(tc.tile_pool(name="iop", bufs=2 * il + 1))
        p2p = ctx.enter_context(tc.tile_pool(name="p2p", bufs=3))
        p2o = ctx.enter_context(tc.tile_pool(name="p2o", bufs=2))
        pp = ctx.enter_context(tc.tile_pool(name="pp", bufs=ppbufs,
                                            space="PSUM"))

        # ---- resident weights + biases ----
        # Weight DMAs are issued inside the phase-1 e loop (just before
        # first use) so the first chains are not stuck behind the whole
        # 3.9 MB preload.
        w1_sb, whA_sb, whB_sb, wo_sb = {}, {}, {}, {}

        def load_weights(e):
            w1_sb[e] = wp.tile([DIN, HID], F32R, tag=f"w1_{e}",
                               name=f"w1_{e}")
            nc.sync.dma_start(w1_sb[e], w1[e])
            for l in range(3):
                whA_sb[l, e] = wp.tile([128, HID], F32R, tag=f"whA_{l}_{e}",
                                       name=f"whA_{l}_{e}")
                whB_sb[l, e] = wp.tile([72, HID], F32R, tag=f"whB_{l}_{e}",
                                       name=f"whB_{l}_{e}")
                nc.sync.dma_start(whA_sb[l, e], wh[l, e, 0:128, :])
                nc.sync.dma_start(whB_sb[l, e], wh[l, e, 128:HID, :])
            woA = wp.tile([128, NOUT], F32R, tag=f"woA_{e}", name=f"woA_{e}")
            woB = wp.tile([72, NOUT], F32R, tag=f"woB_{e}", name=f"woB_{e}")
            nc.sync.dma_start(woA, wo[e, 0:128, :])
            nc.sync.dma_start(woB, wo[e, 128:HID, :])
            wo_sb[e] = (woA, woB)

        b1A = wp.tile([128, ENS], F32, tag="b1A", name="b1A")
        b1B = wp.tile([72, ENS], F32, tag="b1B", name="b1B")
        nc.sync.dma_start(b1A, b1T[0:128, :])
        nc.sync.dma_start(b1B, b1T[128:HID, :])
        bhA, bhB = {}, {}
        for l in range(3):
            bhA[l] = wp.tile([128, ENS], F32, tag=f"bhA{l}", name=f"bhA{l}")
            bhB[l] = wp.tile([72, ENS], F32, tag=f"bhB{l}", name=f"bhB{l}")
            nc.sync.dma_start(bhA[l], bhT[l, 0:128, :])
            nc.sync.dma_start(bhB[l], bhT[l, 128:HID, :])
        bmu2_sb = wp.tile([128, ENS], F32, tag="bmu", name="bmu")
        tb2_sb = wp.tile([128, ENS], F32, tag="tb", name="tb")
        nc.sync.dma_start(bmu2_sb, bmuT)
        nc.sync.dma_start(tb2_sb, tbT)
        k1_sb = wp.tile([4 * NSIG, 1], F32, tag="k1", name="k1")
        k2_sb = wp.tile([4 * NSIG, 1], F32, tag="k2", name="k2")
        nc.sync.dma_start(k1_sb, k1v)
        nc.sync.dma_start(k2_sb, k2v)

        # ---- phase 1: MLP chain, silu table set ----
        # Each PSUM tile is [128, grp]: the grp//T matmul passes of a
        # chain step sit side by side, so ScalarE ops span FD=grp with a
        # per-partition bias that is still exact. `il` chains are issued
        # in lockstep so another chain's matmuls cover each silu latency.
        halves = tuple(slice(i * T, (i + 1) * T) for i in range(grp // T))

        def mlp_layer(lhsA, lhsB, biasA, biasB, rhsA, rhsB, e):
            """One layer for one chain step. lhsA/lhsB: [128,200]/[72,200]
            weight K-halves (lhsB unused for the input layer); rhsA/rhsB:
            previous h as ([K1, grp], [K2, grp]) (rhsB None on input)."""
            psM1 = pp.tile([128, grp], F32, tag="ps", name="psM1")
            psM2 = pp.tile([128, grp], F32, tag="ps", name="psM2")
            for h in halves:
                nc.tensor.matmul(psM1[:, h], lhsT=lhsA[:, 0:128],
                                 rhs=rhsA[:, h], start=True, stop=rhsB is None)
                if rhsB is not None:
                    nc.tensor.matmul(psM1[:, h], lhsT=lhsB[:, 0:128],
                                     rhs=rhsB[:, h], start=False, stop=True)
                nc.tensor.matmul(psM2[0:72, h], lhsT=lhsA[:, 128:HID],
                                 rhs=rhsA[:, h], start=True, stop=rhsB is None)
                if rhsB is not None:
                    nc.tensor.matmul(psM2[0:72, h], lhsT=lhsB[:, 128:HID],
                                     rhs=rhsB[:, h], start=False, stop=True)
            hM1 = iop.tile([128, grp], F32R, tag="hM1", name="hM1")
            hM2 = iop.tile([72, grp], F32R, tag="hM2", name="hM2")
            nc.scalar.activation(hM1, psM1, AF.Silu, bias=biasA[:, e:e + 1])
            nc.scalar.activation(hM2, psM2[0:72, :], AF.Silu,
                                 bias=biasB[:, e:e + 1])
            return hM1, hM2

        warm = pp.tile([128, grp], F32, tag="ps", name="warm")
        load_weights(0)
        for i in range(8):
            nc.tensor.matmul(warm[:, 0:HID], lhsT=w1_sb[0][:, 0:128],
                             rhs=w1_sb[0], start=True, stop=True,
                             skip_group_check=True)

        for e in range(ENS):
            if e > 0:
                load_weights(e)
            for p in range(nstep):
                slots = [bass.ts(p * il + i, grp) for i in range(il)]
                xs = []
                for cs in slots:
                    x_sb = iop.tile([DIN, grp], F32R, tag="x", name="x")
                    nc.sync.dma_start(x_sb, xT[e, :, cs])
                    xs.append(x_sb)

                hs = [mlp_layer(w1_sb[e], None, b1A, b1B, x_sb, None, e)
                      for x_sb in xs]
                for l in range(3):
                    hs = [mlp_layer(whA_sb[l, e], whB_sb[l, e],
                                    bhA[l], bhB[l], hM1, hM2, e)
                          for hM1, hM2 in hs]

                # Per chain: psO rows 0:31 sig, 32:63 mu. (f32r matmuls
                # may only write PSUM partition base 0; PSUM reads with
                # base>0 may span at most 32 partitions.)
                woA, woB = wo_sb[e]
                for cs, (hM1, hM2) in zip(slots, hs):
                    psO = pp.tile([128, grp], F32, tag="ps", name="psO")
                    for h in halves:
                        nc.tensor.matmul(psO[0:NOUT, h], lhsT=woA,
                                         rhs=hM1[:, h], start=True, stop=False)
                        nc.tensor.matmul(psO[0:NOUT, h], lhsT=woB,
                                         rhs=hM2[:, h], start=False, stop=True)

                    gt = iop.tile([NSIG, grp], F32, tag="gt", name="gt")
                    nc.scalar.activation(gt, psO[0:NSIG, :], AF.Tanh,
                                         bias=tb2_sb[0:NSIG, e:e + 1])
                    wv = iop.tile([NSIG, grp], F32, tag="wv1", name="wv1")
                    nc.vector.tensor_scalar(wv, gt, k1_sb[0:NSIG, 0:1],
                                            k2_sb[0:NSIG, 0:1],
                                            ALU.mult, ALU.add)
                    nc.sync.dma_start(gdram[e, :, cs], wv)

                    mu_sb = iop.tile([NSIG, grp], F32, tag="mu", name="mu")
                    nc.vector.tensor_scalar_add(mu_sb, psO[32:32 + NSIG, :],
                                                bmu2_sb[32:32 + NSIG,
                                                        e:e + 1])
                    nc.sync.dma_start(mu_out[e, :, cs], mu_sb)

        # ---- phase 2: sigma = sqrt(w) from the w scratch, sqrt set ----
        # Hard scheduling barrier: phase-2 Sqrt must not interleave with
        # phase-1 Silu/Tanh, or the ACT table set thrashes (~1.3us/reload).
        # Ensembles are packed 4-at-a-time onto partitions (124 lanes).
        tc.strict_bb_all_engine_barrier()
        chunks = [(e0, ne, c) for c in range(npc // P2C)
                  for (e0, ne) in ((0, 4), (4, 3))]
        for e0, ne, c in chunks:
            P = ne * NSIG
            if True:
                ps = bass.ts(c, P2C)
                gin = p2p.tile([4 * NSIG, P2C], F32, tag="gin", name="gin")
                for j in range(ne):
                    nc.sync.dma_start(gin[j * NSIG:(j + 1) * NSIG, :],
                                      gdram[e0 + j, :, ps])
                sg = p2o.tile([4 * NSIG, P2C], F32, tag="sg", name="sg")
                nc.scalar.activation(sg[0:P], gin[0:P], AF.Sqrt)
                for j in range(ne):
                    nc.sync.dma_start(sg_out[e0 + j, :, ps],
                                      sg[j * NSIG:(j + 1) * NSIG, :])

    nc.compile()
    return nc


def kernel(s, a, W1, b1, Wh, bh, Wmu, bmu, Wsig, bsig,
           max_lv_s, min_lv_s, max_lv_r, min_lv_r):
    N = s.shape[0]
    npc = N // NCORES

    f = np.float32
    x = np.concatenate([np.asarray(s, f), np.asarray(a, f)], axis=-1)
    xT = np.ascontiguousarray(np.transpose(x, (1, 2, 0)))      # [E, 38, N]

    maxv = np.concatenate([np.asarray(max_lv_s, f)[0],
                           np.asarray(max_lv_r, f)[0]])        # [31]
    minv = np.concatenate([np.asarray(min_lv_s, f)[0],
                           np.asarray(min_lv_r, f)[0]])        # [31]

    z1 = np.zeros((ENS, HID, 1), f)
    wo = np.ascontiguousarray(
        np.concatenate([np.asarray(Wsig, f), z1, np.asarray(Wmu, f), z1],
                       axis=2))
    b1T = np.ascontiguousarray(np.asarray(b1, f).T)            # [200, 7]
    bhT = np.ascontiguousarray(np.transpose(np.asarray(bh, f), (0, 2, 1)))
    tbT = np.zeros((128, ENS), f)           # tanh bias at rows 0:31, 64:95
    tbT[0:31] = np.asarray(bsig, f).T - maxv[:, None] / 2.0
    tbT[64:95] = tbT[0:31]
    bmuT = np.zeros((128, ENS), f)          # mu bias at rows 32:63, 96:127
    bmuT[32:63] = np.asarray(bmu, f).T
    bmuT[96:127] = bmuT[32:63]
    k1 = (0.5 * np.exp(maxv.astype(np.float64)))[:, None]
    k2 = np.tile((np.exp(minv.astype(np.float64))[:, None] + k1), (4, 1))
    k2 = k2.astype(f)
    k1 = np.tile(k1, (4, 1)).astype(f)

    key = npc
    if key not in _cache:
        _cache[key] = _build(npc)
    nc = _cache[key]

    common = dict(w1=np.ascontiguousarray(np.asarray(W1, f)),
                  wh=np.ascontiguousarray(np.asarray(Wh, f)),
                  wo=wo, b1T=b1T, bhT=bhT, bmuT=bmuT, tbT=tbT,
                  k1v=np.ascontiguousarray(k1), k2v=np.ascontiguousarray(k2))
    in_maps = []
    for c in range(NCORES):
        m = dict(common)
        m["xT"] = np.ascontiguousarray(xT[:, :, c * npc:(c + 1) * npc])
        in_maps.append(m)

    trace = os.environ.get("BASS_KERNEL_TRACE", "0") == "1"
    res = run_bass_kernel_spmd(nc, in_maps, list(range(NCORES)), trace=trace)
    kernel.last_results = res

    mu = np.concatenate([r["mu_out"] for r in res.results], axis=2)
    sg = np.concatenate([r["sg_out"] for r in res.results], axis=2)
    mu = np.transpose(mu, (2, 0, 1))                           # [N, E, 31]
    sg = np.transpose(sg, (2, 0, 1))

    ds_mu = np.ascontiguousarray(mu[:, :, :30])
    r_mu = np.ascontiguousarray(mu[:, :, 30:31])
    ds_sg = np.ascontiguousarray(sg[:, :, :30])
    r_sg = np.ascontiguousarray(sg[:, :, 30:31])
    return ((ds_mu, ds_sg), (r_mu, r_sg))


# revision 24
# speedup vs baseline: 1.0270x; 1.0265x over previous
"""Trainium2 Bass kernel for nn_EnsembleEnvProb (ensemble probabilistic MLP).

Math (per ensemble member e, batch row n):
  x = concat(s, a)                                   [38]
  h = swish(x @ W1[e] + b1[e]); 3x swish(h @ Wh[l,e] + bh[l,e])
  mu    = h @ Wmu[e] + bmu[e]                        [31]
  z     = h @ Wsig[e] + bsig[e]                      [31]  (raw logvar/2)
  soft-clamp z into [min_lv, max_lv] via softplus, sigma = exp(z')

Identity used on-chip (no softplus table exists on TRN2):
  g = sigmoid(2z - max) = 0.5 + 0.5*tanh(z - max/2)
  sigma^2 = exp(min) + 0.5*exp(max)*(1 + tanh(z - max/2))
  sigma = Sqrt(k1*t + k2), k1 = 0.5*exp(max), k2 = exp(min) + k1

Sharding: batch N split across 8 NeuronCores (data parallel, weights
replicated). Host pre-transposes activations to [E, feat, N] so all
on-chip tensors are feature-major (features on SBUF partitions, samples
on the free dim) and every DMA is wide and contiguous.

Per core (N_loc = N/8), phase 1 (silu ACT table set) iterates over
(e, pair-of-512-sample tiles): float32r matmuls with K split 128+72
accumulating in PSUM, hidden M split 128+72; the two tiles of a pair
share PSUM tiles side by side so every ScalarE op runs at FD=1024 with
a correct per-partition bias. mu leaves directly; tanh(z - max/2) goes
to a DRAM scratch. Phase 2 (exp/ln set) finishes sigma.
"""

import os
from contextlib import ExitStack

import numpy as np

import concourse.bass as bass
import concourse.mybir as mybir
import concourse.tile as tile
from concourse import bacc
from concourse.bass_utils import run_bass_kernel_spmd

F32 = mybir.dt.float32
F32R = mybir.dt.float32r
AF = mybir.ActivationFunctionType
ALU = mybir.AluOpType

ENS = 7
DIN = 38
HID = 200
NSIG = 31          # outputs per head (30 ds + 1 r)
NOUT = 64          # sigma head [0:31], pad, mu head [32:63], pad
NCORES = 8
T = 512            # samples per matmul pass (fp32 moving-operand max)
GRP = 1024         # samples per chain step (PSUM tile free dim)
IL = 2             # chains issued in lockstep
P2C = 2048         # phase-2 free-dim chunk

_cache = {}


def _build(npc, grp=GRP, il=IL):
    """Build + compile the per-core Bass program for npc samples/core."""
    nstep = npc // (grp * il)
    ppbufs = max(2, 2 * (8 // (2 * (grp // T))) // 1)
    ppbufs = 8 // (grp // T)
    nc = bacc.Bacc("TRN2", target_bir_lowering=False, debug=False,
                   num_devices=NCORES)

    xT = nc.dram_tensor("xT", [ENS, DIN, npc], F32R, kind="ExternalInput").ap()
    w1 = nc.dram_tensor("w1", [ENS, DIN, HID], F32R, kind="ExternalInput").ap()
    wh = nc.dram_tensor("wh", [3, ENS, HID, HID], F32R,
                        kind="ExternalInput").ap()
    wo = nc.dram_tensor("wo", [ENS, HID, NOUT], F32R,
                        kind="ExternalInput").ap()
    b1T = nc.dram_tensor("b1T", [HID, ENS], F32, kind="ExternalInput").ap()
    bhT = nc.dram_tensor("bhT", [3, HID, ENS], F32, kind="ExternalInput").ap()
    bmuT = nc.dram_tensor("bmuT", [128, ENS], F32, kind="ExternalInput").ap()
    tbT = nc.dram_tensor("tbT", [128, ENS], F32, kind="ExternalInput").ap()
    k1v = nc.dram_tensor("k1v", [4 * NSIG, 1], F32,
                         kind="ExternalInput").ap()
    k2v = nc.dram_tensor("k2v", [4 * NSIG, 1], F32,
                         kind="ExternalInput").ap()

    mu_out = nc.dram_tensor("mu_out", [ENS, NSIG, npc], F32,
                            kind="ExternalOutput").ap()
    sg_out = nc.dram_tensor("sg_out", [2, 4 * NSIG, npc], F32,
                            kind="ExternalOutput").ap()
    gdram = nc.dram_tensor("gscratch", [2, 4 * NSIG, npc], F32).ap()

    with tile.TileContext(nc) as tc, ExitStack() as ctx:
        wp = ctx.enter_context(tc.tile_pool(name="wp", bufs=1))
        iop = ctx.enter_context(tc.tile_pool(name="iop", bufs=2 * il))
        p2p = ctx.enter_context(tc.tile_pool(name="p2p", bufs=2))
        p2o = ctx.enter_context(tc.tile_pool(name="p2o", bufs=2))
        pp = ctx.enter_context(tc.tile_pool(name="pp", bufs=ppbufs,
                                            space="PSUM"))

        # ---- resident weights + biases ----
        # Weight DMAs are issued inside the phase-1 e loop (just before
        # first use) so the first chains are not stuck behind the whole
        # 3.9 MB preload.
        w1_sb, whA_sb, whB_sb, wo_sb = {}, {}, {}, {}

        def load_weights(e):
            w1_sb[e] = wp.tile([DIN, HID], F32R, tag=f"w1_{e}",
                               name=f"w1_{e}")
            nc.sync.dma_start(w1_sb[e], w1[e])
            for l in range(3):
                whA_sb[l, e] = wp.tile([128, HID], F32R, tag=f"whA_{l}_{e}",
                                       name=f"whA_{l}_{e}")
                whB_sb[l, e] = wp.tile([72, HID], F32R, tag=f"whB_{l}_{e}",
                                       name=f"whB_{l}_{e}")
                nc.sync.dma_start(whA_sb[l, e], wh[l, e, 0:128, :])
                nc.sync.dma_start(whB_sb[l, e], wh[l, e, 128:HID, :])
            woA = wp.tile([128, NOUT], F32R, tag=f"woA_{e}", name=f"woA_{e}")
            woB = wp.tile([72, NOUT], F32R, tag=f"woB_{e}", name=f"woB_{e}")
            nc.sync.dma_start(woA, wo[e, 0:128, :])
            nc.sync.dma_start(woB, wo[e, 128:HID, :])
            wo_sb[e] = (woA, woB)

        b1A = wp.tile([128, ENS], F32, tag="b1A", name="b1A")
        b1B = wp.tile([72, ENS], F32, tag="b1B", name="b1B")
        nc.sync.dma_start(b1A, b1T[0:128, :])
        nc.sync.dma_start(b1B, b1T[128:HID, :])
        bhA, bhB = {}, {}
        for l in range(3):
            bhA[l] = wp.tile([128, ENS], F32, tag=f"bhA{l}", name=f"bhA{l}")
            bhB[l] = wp.tile([72, ENS], F32, tag=f"bhB{l}", name=f"bhB{l}")
            nc.sync.dma_start(bhA[l], bhT[l, 0:128, :])
            nc.sync.dma_start(bhB[l], bhT[l, 128:HID, :])
        bmu2_sb = wp.tile([128, ENS], F32, tag="bmu", name="bmu")
        tb2_sb = wp.tile([128, ENS], F32, tag="tb", name="tb")
        nc.sync.dma_start(bmu2_sb, bmuT)
        nc.sync.dma_start(tb2_sb, tbT)
        k1_sb = wp.tile([4 * NSIG, 1], F32, tag="k1", name="k1")
        k2_sb = wp.tile([4 * NSIG, 1], F32, tag="k2", name="k2")
        nc.sync.dma_start(k1_sb, k1v)
        nc.sync.dma_start(k2_sb, k2v)

        # ---- phase 1: MLP chain, silu table set ----
        # Each PSUM tile is [128, grp]: the grp//T matmul passes of a
        # chain step sit side by side, so ScalarE ops span FD=grp with a
        # per-partition bias that is still exact. `il` chains are issued
        # in lockstep so another chain's matmuls cover each silu latency.
        halves = tuple(slice(i * T, (i + 1) * T) for i in range(grp // T))

        def mlp_layer(lhsA, lhsB, biasA, biasB, rhsA, rhsB, e):
            """One layer for one chain step. lhsA/lhsB: [128,200]/[72,200]
            weight K-halves (lhsB unused for the input layer); rhsA/rhsB:
            previous h as ([K1, grp], [K2, grp]) (rhsB None on input)."""
            psM1 = pp.tile([128, grp], F32, tag="ps", name="psM1")
            psM2 = pp.tile([128, grp], F32, tag="ps", name="psM2")
            for h in halves:
                nc.tensor.matmul(psM1[:, h], lhsT=lhsA[:, 0:128],
                                 rhs=rhsA[:, h], start=True, stop=rhsB is None)
                if rhsB is not None:
                    nc.tensor.matmul(psM1[:, h], lhsT=lhsB[:, 0:128],
                                     rhs=rhsB[:, h], start=False, stop=True)
                nc.tensor.matmul(psM2[0:72, h], lhsT=lhsA[:, 128:HID],
                                 rhs=rhsA[:, h], start=True, stop=rhsB is None)
                if rhsB is not None:
                    nc.tensor.matmul(psM2[0:72, h], lhsT=lhsB[:, 128:HID],
                                     rhs=rhsB[:, h], start=False, stop=True)
            hM1 = iop.tile([128, grp], F32R, tag="hM1", name="hM1")
            hM2 = iop.tile([72, grp], F32R, tag="hM2", name="hM2")
            nc.scalar.activation(hM1, psM1, AF.Silu, bias=biasA[:, e:e + 1])
            nc.scalar.activation(hM2, psM2[0:72, :], AF.Silu,
                                 bias=biasB[:, e:e + 1])
            return hM1, hM2

        warm = pp.tile([128, grp], F32, tag="ps", name="warm")
        load_weights(0)
        for i in range(8):
            nc.tensor.matmul(warm[:, 0:HID], lhsT=w1_sb[0][:, 0:128],
                             rhs=w1_sb[0], start=True, stop=True,
                             skip_group_check=True)

        for e in range(ENS):
            if e > 0:
                load_weights(e)
            for p in range(nstep):
                slots = [bass.ts(p * il + i, grp) for i in range(il)]
                x_sb = iop.tile([DIN, il * grp], F32R, tag="x", name="x")
                nc.sync.dma_start(x_sb, xT[e, :, bass.ts(p, il * grp)])
                xs = [x_sb[:, bass.ts(i, grp)] for i in range(il)]

                hs = [mlp_layer(w1_sb[e], None, b1A, b1B, x_sb, None, e)
                      for x_sb in xs]
                for l in range(3):
                    hs = [mlp_layer(whA_sb[l, e], whB_sb[l, e],
                                    bhA[l], bhB[l], hM1, hM2, e)
                          for hM1, hM2 in hs]

                # Per chain: psO rows 0:31 sig, 32:63 mu. (f32r matmuls
                # may only write PSUM partition base 0; PSUM reads with
                # base>0 may span at most 32 partitions.)
                woA, woB = wo_sb[e]
                for cs, (hM1, hM2) in zip(slots, hs):
                    psO = pp.tile([128, grp], F32, tag="ps", name="psO")
                    for h in halves:
                        nc.tensor.matmul(psO[0:NOUT, h], lhsT=woA,
                                         rhs=hM1[:, h], start=True, stop=False)
                        nc.tensor.matmul(psO[0:NOUT, h], lhsT=woB,
                                         rhs=hM2[:, h], start=False, stop=True)

                    gt = iop.tile([NSIG, grp], F32, tag="gt", name="gt")
                    nc.scalar.activation(gt, psO[0:NSIG, :], AF.Tanh,
                                         bias=tb2_sb[0:NSIG, e:e + 1])
                    wv = iop.tile([NSIG, grp], F32, tag="wv1", name="wv1")
                    nc.vector.tensor_scalar(wv, gt, k1_sb[0:NSIG, 0:1],
                                            k2_sb[0:NSIG, 0:1],
                                            ALU.mult, ALU.add)
                    blk, row = divmod(e, 4)
                    nc.sync.dma_start(
                        gdram[blk, row * NSIG:(row + 1) * NSIG, cs], wv)

                    mu_sb = iop.tile([NSIG, grp], F32, tag="mu", name="mu")
                    nc.vector.tensor_scalar_add(mu_sb, psO[32:32 + NSIG, :],
                                                bmu2_sb[32:32 + NSIG,
                                                        e:e + 1])
                    nc.sync.dma_start(mu_out[e, :, cs], mu_sb)

        # ---- phase 2: sigma = sqrt(w) from the w scratch, sqrt set ----
        # Hard scheduling barrier: phase-2 Sqrt must not interleave with
        # phase-1 Silu/Tanh, or the ACT table set thrashes (~1.3us/reload).
        # Ensembles are packed 4-at-a-time onto partitions (124 lanes).
        # All gin loads are issued up front into dedicated buffers so they
        # fan out across the 8 DMA queues; the Sqrts then run gap-free.
        tc.strict_bb_all_engine_barrier()
        pc = 2048
        chunks = [(blk, c) for c in range(npc // pc) for blk in (0, 1)]
        gins = []
        for i, (blk, c) in enumerate(chunks):
            P = 4 * NSIG if blk == 0 else 3 * NSIG
            gin = p2p.tile([4 * NSIG, pc], F32, tag="gin", name="gin")
            nc.sync.dma_start(gin[0:P], gdram[blk, 0:P, bass.ts(c, pc)])
            gins.append(gin)
        for (blk, c), gin in zip(chunks, gins):
            P = 4 * NSIG if blk == 0 else 3 * NSIG
            sg = p2o.tile([4 * NSIG, pc], F32, tag="sg", name="sg")
            nc.scalar.activation(sg[0:P], gin[0:P], AF.Sqrt)
            nc.sync.dma_start(sg_out[blk, 0:P, bass.ts(c, pc)], sg[0:P])

    nc.compile()
    return nc


def kernel(s, a, W1, b1, Wh, bh, Wmu, bmu, Wsig, bsig,
           max_lv_s, min_lv_s, max_lv_r, min_lv_r):
    N = s.shape[0]
    npc = N // NCORES

    f = np.float32
    x = np.concatenate([np.asarray(s, f), np.asarray(a, f)], axis=-1)
    xT = np.ascontiguousarray(np.transpose(x, (1, 2, 0)))      # [E, 38, N]

    maxv = np.concatenate([np.asarray(max_lv_s, f)[0],
                           np.asarray(max_lv_r, f)[0]])        # [31]
    minv = np.concatenate([np.asarray(min_lv_s, f)[0],
                           np.asarray(min_lv_r, f)[0]])        # [31]

    z1 = np.zeros((ENS, HID, 1), f)
    wo = np.ascontiguousarray(
        np.concatenate([np.asarray(Wsig, f), z1, np.asarray(Wmu, f), z1],
                       axis=2))
    b1T = np.ascontiguousarray(np.asarray(b1, f).T)            # [200, 7]
    bhT = np.ascontiguousarray(np.transpose(np.asarray(bh, f), (0, 2, 1)))
    tbT = np.zeros((128, ENS), f)           # tanh bias at rows 0:31, 64:95
    tbT[0:31] = np.asarray(bsig, f).T - maxv[:, None] / 2.0
    tbT[64:95] = tbT[0:31]
    bmuT = np.zeros((128, ENS), f)          # mu bias at rows 32:63, 96:127
    bmuT[32:63] = np.asarray(bmu, f).T
    bmuT[96:127] = bmuT[32:63]
    k1 = (0.5 * np.exp(maxv.astype(np.float64)))[:, None]
    k2 = np.tile((np.exp(minv.astype(np.float64))[:, None] + k1), (4, 1))
    k2 = k2.astype(f)
    k1 = np.tile(k1, (4, 1)).astype(f)

    key = npc
    if key not in _cache:
        _cache[key] = _build(npc)
    nc = _cache[key]

    common = dict(w1=np.ascontiguousarray(np.asarray(W1, f)),
                  wh=np.ascontiguousarray(np.asarray(Wh, f)),
                  wo=wo, b1T=b1T, bhT=bhT, bmuT=bmuT, tbT=tbT,
                  k1v=np.ascontiguousarray(k1), k2v=np.ascontiguousarray(k2))
    in_maps = []
    for c in range(NCORES):
        m = dict(common)
        m["xT"] = np.ascontiguousarray(xT[:, :, c * npc:(c + 1) * npc])
        in_maps.append(m)

    trace = os.environ.get("BASS_KERNEL_TRACE", "0") == "1"
    res = run_bass_kernel_spmd(nc, in_maps, list(range(NCORES)), trace=trace)
    kernel.last_results = res

    mu = np.concatenate([r["mu_out"] for r in res.results], axis=2)
    sgp = np.concatenate([r["sg_out"] for r in res.results], axis=2)
    sg = np.empty((ENS, NSIG, N), np.float32)
    for e in range(ENS):
        blk, row = divmod(e, 4)
        sg[e] = sgp[blk, row * NSIG:(row + 1) * NSIG, :]
    mu = np.transpose(mu, (2, 0, 1))                           # [N, E, 31]
    sg = np.transpose(sg, (2, 0, 1))

    ds_mu = np.ascontiguousarray(mu[:, :, :30])
    r_mu = np.ascontiguousarray(mu[:, :, 30:31])
    ds_sg = np.ascontiguousarray(sg[:, :, :30])
    r_sg = np.ascontiguousarray(sg[:, :, 30:31])
    return ((ds_mu, ds_sg), (r_mu, r_sg))


# revision 25
# speedup vs baseline: 1.0552x; 1.0275x over previous
"""Trainium2 Bass kernel for nn_EnsembleEnvProb (ensemble probabilistic MLP).

Math (per ensemble member e, batch row n):
  x = concat(s, a)                                   [38]
  h = swish(x @ W1[e] + b1[e]); 3x swish(h @ Wh[l,e] + bh[l,e])
  mu    = h @ Wmu[e] + bmu[e]                        [31]
  z     = h @ Wsig[e] + bsig[e]                      [31]  (raw logvar/2)
  soft-clamp z into [min_lv, max_lv] via softplus, sigma = exp(z')

Identity used on-chip (no softplus table exists on TRN2):
  g = sigmoid(2z - max) = 0.5 + 0.5*tanh(z - max/2)
  sigma^2 = exp(min) + 0.5*exp(max)*(1 + tanh(z - max/2))
  sigma = Sqrt(k1*t + k2), k1 = 0.5*exp(max), k2 = exp(min) + k1

Sharding: batch N split across 8 NeuronCores (data parallel, weights
replicated). Host pre-transposes activations to [E, feat, N] so all
on-chip tensors are feature-major (features on SBUF partitions, samples
on the free dim) and every DMA is wide and contiguous.

Per core (N_loc = N/8), phase 1 (silu ACT table set) iterates over
(e, pair-of-512-sample tiles): float32r matmuls with K split 128+72
accumulating in PSUM, hidden M split 128+72; the two tiles of a pair
share PSUM tiles side by side so every ScalarE op runs at FD=1024 with
a correct per-partition bias. mu leaves directly; tanh(z - max/2) goes
to a DRAM scratch. Phase 2 (exp/ln set) finishes sigma.
"""

import os
from contextlib import ExitStack

import numpy as np

import concourse.bass as bass
import concourse.mybir as mybir
import concourse.tile as tile
from concourse import bacc
from concourse.bass_utils import run_bass_kernel_spmd

F32 = mybir.dt.float32
F32R = mybir.dt.float32r
AF = mybir.ActivationFunctionType
ALU = mybir.AluOpType

ENS = 7
DIN = 38
HID = 200
NSIG = 31          # outputs per head (30 ds + 1 r)
NOUT = 64          # sigma head [0:31], pad, mu head [32:63], pad
NCORES = 8
T = 512            # samples per matmul pass (fp32 moving-operand max)
GRP = 1024         # samples per chain step (PSUM tile free dim)
IL = 2             # chains issued in lockstep
P2C = 2048         # phase-2 free-dim chunk

_cache = {}


def _build(npc, grp=GRP, il=IL):
    """Build + compile the per-core Bass program for npc samples/core."""
    nstep = npc // (grp * il)
    ppbufs = max(2, 2 * (8 // (2 * (grp // T))) // 1)
    ppbufs = 8 // (grp // T)
    nc = bacc.Bacc("TRN2", target_bir_lowering=False, debug=False,
                   num_devices=NCORES)

    xT = nc.dram_tensor("xT", [ENS, DIN, npc], F32R, kind="ExternalInput").ap()
    w1 = nc.dram_tensor("w1", [ENS, DIN, HID], F32R, kind="ExternalInput").ap()
    wh = nc.dram_tensor("wh", [3, ENS, HID, HID], F32R,
                        kind="ExternalInput").ap()
    wo = nc.dram_tensor("wo", [ENS, HID, NOUT], F32R,
                        kind="ExternalInput").ap()
    biasall = nc.dram_tensor("biasall", [128, 72], F32,
                             kind="ExternalInput").ap()

    mu_out = nc.dram_tensor("mu_out", [ENS, NSIG, npc], F32,
                            kind="ExternalOutput").ap()
    sg_out = nc.dram_tensor("sg_out", [2, 4 * NSIG, npc], F32,
                            kind="ExternalOutput").ap()
    gdram = nc.dram_tensor("gscratch", [2, 4 * NSIG, npc], F32).ap()

    with tile.TileContext(nc) as tc, ExitStack() as ctx:
        wp = ctx.enter_context(tc.tile_pool(name="wp", bufs=1))
        iop = ctx.enter_context(tc.tile_pool(name="iop", bufs=2 * il))
        p2p = ctx.enter_context(tc.tile_pool(name="p2p", bufs=2))
        p2o = ctx.enter_context(tc.tile_pool(name="p2o", bufs=2))
        pp = ctx.enter_context(tc.tile_pool(name="pp", bufs=ppbufs,
                                            space="PSUM"))

        # ---- resident weights + biases ----
        # Weight DMAs are issued inside the phase-1 e loop (just before
        # first use) so the first chains are not stuck behind the whole
        # 3.9 MB preload.
        w1_sb, whA_sb, whB_sb, wo_sb = {}, {}, {}, {}

        def load_weights(e):
            w1_sb[e] = wp.tile([DIN, HID], F32R, tag=f"w1_{e}",
                               name=f"w1_{e}")
            nc.sync.dma_start(w1_sb[e], w1[e])
            for l in range(3):
                whA_sb[l, e] = wp.tile([128, HID], F32R, tag=f"whA_{l}_{e}",
                                       name=f"whA_{l}_{e}")
                whB_sb[l, e] = wp.tile([72, HID], F32R, tag=f"whB_{l}_{e}",
                                       name=f"whB_{l}_{e}")
                nc.sync.dma_start(whA_sb[l, e], wh[l, e, 0:128, :])
                nc.sync.dma_start(whB_sb[l, e], wh[l, e, 128:HID, :])
            woA = wp.tile([128, NOUT], F32R, tag=f"woA_{e}", name=f"woA_{e}")
            woB = wp.tile([72, NOUT], F32R, tag=f"woB_{e}", name=f"woB_{e}")
            nc.sync.dma_start(woA, wo[e, 0:128, :])
            nc.sync.dma_start(woB, wo[e, 128:HID, :])
            wo_sb[e] = (woA, woB)

        # First x-load issued before anything else so the pipeline fills
        # immediately; all 13 bias vectors arrive as ONE packed DMA.
        x0 = iop.tile([DIN, il * grp], F32R, tag="x", name="x0")
        nc.sync.dma_start(x0, xT[0, :, 0:il * grp])
        ba = wp.tile([128, 72], F32, tag="ba", name="ba")
        nc.sync.dma_start(ba, biasall)
        b1A = ba[:, 0:7]
        bhA = {l: ba[:, 7 + 7 * l:14 + 7 * l] for l in range(3)}
        bmu2_sb = ba[:, 28:35]
        tb2_sb = ba[:, 35:42]
        b1B = ba[0:72, 42:49]
        bhB = {l: ba[0:72, 49 + 7 * l:56 + 7 * l] for l in range(3)}
        k1_sb = ba[0:4 * NSIG, 70:71]
        k2_sb = ba[0:4 * NSIG, 71:72]

        # ---- phase 1: MLP chain, silu table set ----
        # Each PSUM tile is [128, grp]: the grp//T matmul passes of a
        # chain step sit side by side, so ScalarE ops span FD=grp with a
        # per-partition bias that is still exact. `il` chains are issued
        # in lockstep so another chain's matmuls cover each silu latency.
        halves = tuple(slice(i * T, (i + 1) * T) for i in range(grp // T))

        def mlp_layer(lhsA, lhsB, biasA, biasB, rhsA, rhsB, e):
            """One layer for one chain step. lhsA/lhsB: [128,200]/[72,200]
            weight K-halves (lhsB unused for the input layer); rhsA/rhsB:
            previous h as ([K1, grp], [K2, grp]) (rhsB None on input)."""
            psM1 = pp.tile([128, grp], F32, tag="ps", name="psM1")
            psM2 = pp.tile([128, grp], F32, tag="ps", name="psM2")
            for h in halves:
                nc.tensor.matmul(psM1[:, h], lhsT=lhsA[:, 0:128],
                                 rhs=rhsA[:, h], start=True, stop=rhsB is None)
                if rhsB is not None:
                    nc.tensor.matmul(psM1[:, h], lhsT=lhsB[:, 0:128],
                                     rhs=rhsB[:, h], start=False, stop=True)
                nc.tensor.matmul(psM2[0:72, h], lhsT=lhsA[:, 128:HID],
                                 rhs=rhsA[:, h], start=True, stop=rhsB is None)
                if rhsB is not None:
                    nc.tensor.matmul(psM2[0:72, h], lhsT=lhsB[:, 128:HID],
                                     rhs=rhsB[:, h], start=False, stop=True)
            hM1 = iop.tile([128, grp], F32R, tag="hM1", name="hM1")
            hM2 = iop.tile([72, grp], F32R, tag="hM2", name="hM2")
            nc.scalar.activation(hM1, psM1, AF.Silu, bias=biasA[:, e:e + 1])
            nc.scalar.activation(hM2, psM2[0:72, :], AF.Silu,
                                 bias=biasB[:, e:e + 1])
            return hM1, hM2

        warm = pp.tile([128, grp], F32, tag="ps", name="warm")
        load_weights(0)
        for i in range(8):
            nc.tensor.matmul(warm[:, 0:HID], lhsT=w1_sb[0][:, 0:128],
                             rhs=w1_sb[0], start=True, stop=True,
                             skip_group_check=True)

        for e in range(ENS):
            if e > 0:
                load_weights(e)
            for p in range(nstep):
                slots = [bass.ts(p * il + i, grp) for i in range(il)]
                if e == 0 and p == 0:
                    x_sb = x0
                else:
                    x_sb = iop.tile([DIN, il * grp], F32R, tag="x", name="x")
                    nc.sync.dma_start(x_sb, xT[e, :, bass.ts(p, il * grp)])
                xs = [x_sb[:, bass.ts(i, grp)] for i in range(il)]

                hs = [mlp_layer(w1_sb[e], None, b1A, b1B, x_sb, None, e)
                      for x_sb in xs]
                for l in range(3):
                    hs = [mlp_layer(whA_sb[l, e], whB_sb[l, e],
                                    bhA[l], bhB[l], hM1, hM2, e)
                          for hM1, hM2 in hs]

                # Per chain: psO rows 0:31 sig, 32:63 mu. (f32r matmuls
                # may only write PSUM partition base 0; PSUM reads with
                # base>0 may span at most 32 partitions.)
                woA, woB = wo_sb[e]
                for cs, (hM1, hM2) in zip(slots, hs):
                    psO = pp.tile([128, grp], F32, tag="ps", name="psO")
                    for h in halves:
                        nc.tensor.matmul(psO[0:NOUT, h], lhsT=woA,
                                         rhs=hM1[:, h], start=True, stop=False)
                        nc.tensor.matmul(psO[0:NOUT, h], lhsT=woB,
                                         rhs=hM2[:, h], start=False, stop=True)

                    gt = iop.tile([NSIG, grp], F32, tag="gt", name="gt")
                    nc.scalar.activation(gt, psO[0:NSIG, :], AF.Tanh,
                                         bias=tb2_sb[0:NSIG, e:e + 1])
                    wv = iop.tile([NSIG, grp], F32, tag="wv1", name="wv1")
                    nc.vector.tensor_scalar(wv, gt, k1_sb[0:NSIG, 0:1],
                                            k2_sb[0:NSIG, 0:1],
                                            ALU.mult, ALU.add)
                    blk, row = divmod(e, 4)
                    nc.sync.dma_start(
                        gdram[blk, row * NSIG:(row + 1) * NSIG, cs], wv)

                    mu_sb = iop.tile([NSIG, grp], F32, tag="mu", name="mu")
                    nc.vector.tensor_scalar_add(mu_sb, psO[32:32 + NSIG, :],
                                                bmu2_sb[32:32 + NSIG,
                                                        e:e + 1])
                    nc.sync.dma_start(mu_out[e, :, cs], mu_sb)

        # ---- phase 2: sigma = sqrt(w) from the w scratch, sqrt set ----
        # Hard scheduling barrier: phase-2 Sqrt must not interleave with
        # phase-1 Silu/Tanh, or the ACT table set thrashes (~1.3us/reload).
        # Ensembles are packed 4-at-a-time onto partitions (124 lanes).
        # All gin loads are issued up front into dedicated buffers so they
        # fan out across the 8 DMA queues; the Sqrts then run gap-free.
        tc.strict_bb_all_engine_barrier()
        pc = 2048
        chunks = [(blk, c) for c in range(npc // pc) for blk in (0, 1)]
        gins = []
        for i, (blk, c) in enumerate(chunks):
            P = 4 * NSIG if blk == 0 else 3 * NSIG
            gin = p2p.tile([4 * NSIG, pc], F32, tag="gin", name="gin")
            nc.sync.dma_start(gin[0:P], gdram[blk, 0:P, bass.ts(c, pc)])
            gins.append(gin)
        for (blk, c), gin in zip(chunks, gins):
            P = 4 * NSIG if blk == 0 else 3 * NSIG
            sg = p2o.tile([4 * NSIG, pc], F32, tag="sg", name="sg")
            nc.scalar.activation(sg[0:P], gin[0:P], AF.Sqrt)
            nc.sync.dma_start(sg_out[blk, 0:P, bass.ts(c, pc)], sg[0:P])

    nc.compile()
    return nc


def kernel(s, a, W1, b1, Wh, bh, Wmu, bmu, Wsig, bsig,
           max_lv_s, min_lv_s, max_lv_r, min_lv_r):
    N = s.shape[0]
    npc = N // NCORES

    f = np.float32
    x = np.concatenate([np.asarray(s, f), np.asarray(a, f)], axis=-1)
    xT = np.ascontiguousarray(np.transpose(x, (1, 2, 0)))      # [E, 38, N]

    maxv = np.concatenate([np.asarray(max_lv_s, f)[0],
                           np.asarray(max_lv_r, f)[0]])        # [31]
    minv = np.concatenate([np.asarray(min_lv_s, f)[0],
                           np.asarray(min_lv_r, f)[0]])        # [31]

    z1 = np.zeros((ENS, HID, 1), f)
    wo = np.ascontiguousarray(
        np.concatenate([np.asarray(Wsig, f), z1, np.asarray(Wmu, f), z1],
                       axis=2))
    b1T = np.ascontiguousarray(np.asarray(b1, f).T)            # [200, 7]
    bhT = np.ascontiguousarray(np.transpose(np.asarray(bh, f), (0, 2, 1)))
    tbT = np.zeros((128, ENS), f)           # tanh bias at rows 0:31, 64:95
    tbT[0:31] = np.asarray(bsig, f).T - maxv[:, None] / 2.0
    tbT[64:95] = tbT[0:31]
    bmuT = np.zeros((128, ENS), f)          # mu bias at rows 32:63, 96:127
    bmuT[32:63] = np.asarray(bmu, f).T
    bmuT[96:127] = bmuT[32:63]
    k1 = (0.5 * np.exp(maxv.astype(np.float64)))[:, None]
    k2 = np.tile((np.exp(minv.astype(np.float64))[:, None] + k1), (4, 1))
    k2 = k2.astype(f)
    k1 = np.tile(k1, (4, 1)).astype(f)
    biasall = np.zeros((128, 72), f)
    biasall[:, 0:7] = b1T[0:128]
    for l in range(3):
        biasall[:, 7 + 7 * l:14 + 7 * l] = bhT[l, 0:128]
        biasall[0:72, 49 + 7 * l:56 + 7 * l] = bhT[l, 128:HID]
    biasall[:, 28:35] = bmuT
    biasall[:, 35:42] = tbT
    biasall[0:72, 42:49] = b1T[128:HID]
    biasall[0:4 * NSIG, 70:71] = k1
    biasall[0:4 * NSIG, 71:72] = k2

    key = npc
    if key not in _cache:
        _cache[key] = _build(npc)
    nc = _cache[key]

    common = dict(w1=np.ascontiguousarray(np.asarray(W1, f)),
                  wh=np.ascontiguousarray(np.asarray(Wh, f)),
                  wo=wo, biasall=biasall)
    in_maps = []
    for c in range(NCORES):
        m = dict(common)
        m["xT"] = np.ascontiguousarray(xT[:, :, c * npc:(c + 1) * npc])
        in_maps.append(m)

    trace = os.environ.get("BASS_KERNEL_TRACE", "0") == "1"
    res = run_bass_kernel_spmd(nc, in_maps, list(range(NCORES)), trace=trace)
    kernel.last_results = res

    mu = np.concatenate([r["mu_out"] for r in res.results], axis=2)
    sgp = np.concatenate([r["sg_out"] for r in res.results], axis=2)
    sg = np.empty((ENS, NSIG, N), np.float32)
    for e in range(ENS):
        blk, row = divmod(e, 4)
        sg[e] = sgp[blk, row * NSIG:(row + 1) * NSIG, :]
    mu = np.transpose(mu, (2, 0, 1))                           # [N, E, 31]
    sg = np.transpose(sg, (2, 0, 1))

    ds_mu = np.ascontiguousarray(mu[:, :, :30])
    r_mu = np.ascontiguousarray(mu[:, :, 30:31])
    ds_sg = np.ascontiguousarray(sg[:, :, :30])
    r_sg = np.ascontiguousarray(sg[:, :, 30:31])
    return ((ds_mu, ds_sg), (r_mu, r_sg))
